# revision 1
# baseline (speedup 1.0000x reference)
"""2-layer GAT on 8 NeuronCores (Trainium2, Bass/Tile).

Strategy: dst-node sharding. Each core owns 6250 dst nodes arranged into 49
degree-balanced tiles of 128 slots. Per-node feature records
[el fp16(4) | ee-scratch(4) | z fp16(256) | pad(120)] live in DRAM in a
per-core permuted order (own shard first, so er reads are dense). Edge
messages are gathered with dma_gather (int16 indices, low/high split at
32768), scaled by softmax weights, and aggregated per dst tile with a
one-hot matmul on the tensor engine accumulating [s | U] in PSUM.
Layer-2 records are exchanged with an AllGather.
"""
import numpy as np

N = 50000
E = 800000
IN_F, HID, OUT, HEADS = 128, 64, 64, 4
D1 = HEADS * HID   # 256
D2 = HEADS * OUT   # 256
NEG = 0.2
NCORES = 8
SHN = N // NCORES          # 6250 dst nodes per core
TILES = 49
SH = TILES * 128           # 6272 slots per core
NP = NCORES * SH           # 50176 permuted rows
NT0 = NP // 128            # 392 node tiles in phase 0
RECW = 384                 # record width (fp16 cols); el 0:4, ee 4:8, z 8:264
SPLIT = 32768


def _host_prep(x, src, dst, W1, al1, ar1, b1, W2, al2, ar2, b2):
    """Pure-numpy preprocessing: per-core permutations, edge chunking, consts."""
    f32, f16 = np.float32, np.float16
    deg = np.bincount(dst, minlength=N)

    # per-core tile assignment (degree balanced round robin)
    slot_of = np.full(N, -1, np.int64)
    node_of_slot = np.full((NCORES, SH), -1, np.int64)
    for c in range(NCORES):
        nodes = np.arange(c * SHN, (c + 1) * SHN)
        order = nodes[np.argsort(-deg[nodes], kind="stable")]
        i = np.arange(order.size)
        s = (i % TILES) * 128 + i // TILES
        slot_of[order] = s
        node_of_slot[c, s] = order

    # per-core perms of length NP: own shard slots first, then other nodes
    perm_pos = np.zeros((NCORES, N), np.int64)
    node_of_perm = np.full((NCORES, NP), -1, np.int64)
    for c in range(NCORES):
        node_of_perm[c, :SH] = node_of_slot[c]
        others = np.concatenate([np.arange(0, c * SHN),
                                 np.arange((c + 1) * SHN, N)])
        node_of_perm[c, SH:SH + others.size] = others
        own = node_of_slot[c]
        perm_pos[c, own[own >= 0]] = np.flatnonzero(own >= 0)
        perm_pos[c, others] = SH + np.arange(others.size)

    # global (L2) perm row of node n: split-AllGather layout
    # AG#1 gathers shard rows 0:3072 (tiles 0:24) -> rows c*3072 + s
    # AG#2 gathers shard rows 3072:6272 -> rows 8*3072 + c*3200 + (s-3072)
    AG1 = 24 * 128
    cn_ = np.arange(N) // SHN
    gperm_pos = np.where(
        slot_of < AG1,
        cn_ * AG1 + slot_of,
        NCORES * AG1 + cn_ * (SH - AG1) + (slot_of - AG1))

    ecore = dst // SHN
    etile = slot_of[dst] // 128
    edstl = slot_of[dst] % 128
    per = {}
    for c in range(NCORES):
        sel = np.flatnonzero(ecore == c)
        per_tile = etile[sel]
        for t in range(TILES):
            m = sel[per_tile == t]
            per[(c, t)] = (perm_pos[c, src[m]], gperm_pos[src[m]], edstl[m])

    def max_chunks(which):
        nlo = nhi = 0
        for (c, t), (s1, s2, dl) in per.items():
            si = s1 if which == 1 else s2
            nlo = max(nlo, int((si < SPLIT).sum()))
            nhi = max(nhi, int((si >= SPLIT).sum()))
        return -(-nlo // 128), -(-nhi // 128)

    T1L, T1H = max_chunks(1)
    T2L, T2H = max_chunks(2)

    def build(which, TL, TH):
        T = TL + TH
        idx_lo = np.zeros((NCORES, TILES, 128, TL * 8), np.int16)
        idx_hi = np.zeros((NCORES, TILES, 128, TH * 8), np.int16)
        dstl = np.full((NCORES, TILES, 128, T), -1.0, np.float32)
        for (c, t), (s1, s2, dl) in per.items():
            si = s1 if which == 1 else s2
            lo = si < SPLIT
            for mask, TT, base, off, tgt in (
                    (lo, TL, 0, 0, idx_lo), (~lo, TH, SPLIT, TL, idx_hi)):
                vals = (si[mask] - base).astype(np.int16)
                dls = dl[mask].astype(np.float32)
                nlive = vals.size
                padded = np.zeros(TT * 128, np.int16)
                padded[:nlive] = vals
                dpad = np.full(TT * 128, -1.0, np.float32)
                dpad[:nlive] = dls
                ii = np.arange(TT * 128)
                w = np.zeros((16, TT * 8), np.int16)
                w[ii % 16, ii // 16] = padded
                tgt[c, t] = np.tile(w, (8, 1))
                dstl[c, t, ii % 128, off + ii // 128] = dpad
        return idx_lo, idx_hi, dstl

    idx1_lo, idx1_hi, dstl1 = build(1, T1L, T1H)
    idx2_lo, idx2_hi, dstl2 = build(2, T2L, T2H)

    xT = np.ascontiguousarray(x.T).astype(f16)          # [128, N]
    xTp = np.zeros((NCORES, IN_F, NP), f16)
    for c in range(NCORES):
        valid = node_of_perm[c] >= 0
        xTp[c][:, valid] = xT[:, node_of_perm[c][valid]]

    cl1 = np.einsum("khd,hd->kh", W1.reshape(IN_F, HEADS, HID), al1)
    cr1 = np.einsum("khd,hd->kh", W1.reshape(IN_F, HEADS, HID), ar1)
    cw1 = np.concatenate([cl1, cr1], 1).astype(f16)
    cl2 = np.einsum("khd,hd->kh", W2.reshape(D1, HEADS, OUT), al2)
    cr2 = np.einsum("khd,hd->kh", W2.reshape(D1, HEADS, OUT), ar2)
    cw2 = np.concatenate([cl2, cr2], 1).astype(f16)
    b1_tile = np.broadcast_to(b1.astype(f32), (128, D1)).copy()
    b2m = b2.reshape(HEADS, OUT).mean(0).astype(f32)
    b2m_tile = np.broadcast_to(b2m, (128, OUT)).copy()

    consts = dict(
        W1f=W1.astype(f16), cw1=cw1, W2f=W2.astype(f16), cw2=cw2,
        b1_tile=b1_tile, b2m_tile=b2m_tile,
    )
    per_core = []
    for c in range(NCORES):
        per_core.append(dict(
            xTp=xTp[c],
            idx1_lo=idx1_lo[c].reshape(TILES * 128, T1L * 8),
            idx1_hi=idx1_hi[c].reshape(TILES * 128, T1H * 8),
            dstl1=dstl1[c].reshape(TILES * 128, T1L + T1H),
            idx2_lo=idx2_lo[c].reshape(TILES * 128, T2L * 8),
            idx2_hi=idx2_hi[c].reshape(TILES * 128, T2H * 8),
            dstl2=dstl2[c].reshape(TILES * 128, T2L + T2H),
        ))
    meta = dict(T1L=T1L, T1H=T1H, T2L=T2L, T2H=T2H,
                node_of_slot=node_of_slot)
    return consts, per_core, meta


def _build_kernel(T1L, T1H, T2L, T2H, phases=("p0", "l1", "ag", "l2")):
    import concourse.mybir as mybir
    from concourse import bacc
    from concourse.tile import TileContext
    from concourse.masks import make_identity
    dt = mybir.dt
    AF = mybir.ActivationFunctionType
    OP = mybir.AluOpType
    T1, T2 = T1L + T1H, T2L + T2H

    import os as _os
    nc = bacc.Bacc()

    xTp = nc.dram_tensor("xTp", [IN_F, NP], dt.float16, kind="ExternalInput")
    W1f = nc.dram_tensor("W1f", [IN_F, D1], dt.float16, kind="ExternalInput")
    cw1 = nc.dram_tensor("cw1", [IN_F, 8], dt.float16, kind="ExternalInput")
    W2f = nc.dram_tensor("W2f", [D1, D2], dt.float16, kind="ExternalInput")
    cw2 = nc.dram_tensor("cw2", [D1, 8], dt.float16, kind="ExternalInput")
    b1_tile = nc.dram_tensor("b1_tile", [128, D1], dt.float32, kind="ExternalInput")
    b2m_tile = nc.dram_tensor("b2m_tile", [128, OUT], dt.float32, kind="ExternalInput")
    idx1_lo = nc.dram_tensor("idx1_lo", [TILES * 128, T1L * 8], dt.int16, kind="ExternalInput")
    idx1_hi = nc.dram_tensor("idx1_hi", [TILES * 128, T1H * 8], dt.int16, kind="ExternalInput")
    dstl1 = nc.dram_tensor("dstl1", [TILES * 128, T1], dt.float32, kind="ExternalInput")
    idx2_lo = nc.dram_tensor("idx2_lo", [TILES * 128, T2L * 8], dt.int16, kind="ExternalInput")
    idx2_hi = nc.dram_tensor("idx2_hi", [TILES * 128, T2H * 8], dt.int16, kind="ExternalInput")
    dstl2 = nc.dram_tensor("dstl2", [TILES * 128, T2], dt.float32, kind="ExternalInput")
    out = nc.dram_tensor("out", [SH, OUT], dt.float32, kind="ExternalOutput")

    recs1 = nc.dram_tensor("recs1", [NP, RECW], dt.float16, kind="Internal")
    recs2s = nc.dram_tensor("recs2s", [SH, RECW], dt.float16, kind="Internal")
    recs2f = nc.dram_tensor("recs2f", [NP, RECW], dt.float16, kind="Internal",
                            addr_space="Shared")

    # persistent SBUF constants (outside Tile pools; live whole program)
    _cms = []

    def const_tile(shape, dtype):
        cm = nc.sbuf_tensor(shape, dtype)
        t = cm.__enter__()
        _cms.append(cm)
        return t

    W1sb = const_tile([IN_F, D1], dt.float16)
    cw1sb = const_tile([IN_F, 8], dt.float16)
    W2sb0 = const_tile([128, D2], dt.float16)
    W2sb1 = const_tile([128, D2], dt.float16)
    cw2sb0 = const_tile([128, 8], dt.float16)
    cw2sb1 = const_tile([128, 8], dt.float16)
    b1sb = const_tile([128, D1], dt.float32)
    b2msb = const_tile([128, OUT], dt.float32)
    iotaF = const_tile([128, 128], dt.float16)
    ident16 = const_tile([128, 128], dt.float16)
    ident32 = const_tile([128, 128], dt.float32)
    out_sb = const_tile([128, TILES, OUT], dt.float32)
    er1_sb = const_tile([128, TILES, 4], dt.float16)
    er2_sb = const_tile([128, TILES, 4], dt.float16)
    il1_sb = const_tile([128, TILES, T1L * 8], dt.int16)
    ih1_sb = const_tile([128, TILES, T1H * 8], dt.int16)
    dl1_sb = const_tile([128, TILES, T1], dt.float32)
    il2_sb = const_tile([128, TILES, T2L * 8], dt.int16)
    ih2_sb = const_tile([128, TILES, T2H * 8], dt.int16)
    dl2_sb = const_tile([128, TILES, T2], dt.float32)

    # ---------------- Phase 0: layer-1 records ----------------
    MERGE = _os.environ.get("GAT_MERGE", "0") == "1"
    _tc_ctx = TileContext(nc)
    tc = _tc_ctx.__enter__()
    _tc_open = [True]

    def _tc_close():
        if _tc_open[0]:
            _tc_ctx.__exit__(None, None, None)
            _tc_open[0] = False
    if True:
        with (tc.tile_pool(name="init", bufs=1) as ip,
              tc.tile_pool(name="p0", bufs=3) as p0,
              tc.tile_pool(name="p0ps", bufs=2, space="PSUM") as p0ps):
            nc.sync.dma_start(W1sb[:], W1f[:])
            nc.sync.dma_start(cw1sb[:], cw1[:])
            nc.sync.dma_start(W2sb0[:], W2f[0:128, :])
            nc.sync.dma_start(W2sb1[:], W2f[128:256, :])
            nc.sync.dma_start(cw2sb0[:], cw2[0:128, :])
            nc.sync.dma_start(cw2sb1[:], cw2[128:256, :])
            nc.sync.dma_start(b1sb[:], b1_tile[:])
            nc.sync.dma_start(b2msb[:], b2m_tile[:])
            iF32 = ip.tile([128, 128], dt.int32)
            nc.gpsimd.iota(iF32[:], pattern=[[1, 128]], base=0,
                           channel_multiplier=0)
            nc.vector.tensor_copy(iotaF[:], iF32[:])
            make_identity(nc, ident16[:])
            make_identity(nc, ident32[:])

            # stage all edge-phase indices once
            def stage(dst_sb, src_dram, w):
                nc.sync.dma_start(
                    dst_sb[:],
                    src_dram[:].rearrange("(t p) w -> p t w", p=128))
            stage(il1_sb, idx1_lo, T1L * 8)
            stage(ih1_sb, idx1_hi, T1H * 8)
            stage(dl1_sb, dstl1, T1)
            stage(il2_sb, idx2_lo, T2L * 8)
            stage(ih2_sb, idx2_hi, T2H * 8)
            stage(dl2_sb, dstl2, T2)
            B0 = 4
            for gdx in range(NT0 // B0):
                xt = p0.tile([128, B0 * 128], dt.float16, tag="xt")
                nc.sync.dma_start(xt[:], xTp[:, gdx * B0 * 128:(gdx + 1) * B0 * 128])
                zps = p0ps.tile([128, B0 * D1], dt.float32, tag="zps", bufs=2)
                eps = p0ps.tile([128, B0 * 8], dt.float32, tag="eps")
                for j in range(B0):
                    nc.tensor.matmul(out=zps[:, j * D1:(j + 1) * D1],
                                     lhsT=xt[:, j * 128:(j + 1) * 128],
                                     rhs=W1sb[:], start=True, stop=True)
                    nc.tensor.matmul(out=eps[:, j * 8:(j + 1) * 8],
                                     lhsT=xt[:, j * 128:(j + 1) * 128],
                                     rhs=cw1sb[:], start=True, stop=True)
                rec = p0.tile([128, B0, RECW], dt.float16, tag="rec")
                nc.scalar.copy(
                    rec[:, :, 8:8 + D1],
                    zps[:].rearrange("p (b d) -> p b d", b=B0))
                nc.vector.tensor_copy(
                    rec[:, :, 0:4],
                    eps[:].rearrange("p (b d) -> p b d", b=B0)[:, :, 0:4])
                nc.sync.dma_start(
                    recs1[gdx * B0 * 128:(gdx + 1) * B0 * 128, :].rearrange(
                        "(b p) w -> p b w", p=128),
                    rec[:])
                for j in range(B0):
                    t = gdx * B0 + j
                    if t < TILES:
                        nc.vector.tensor_copy(
                            er1_sb[:, t, :],
                            eps[:, j * 8 + 4:j * 8 + 8])

    # ---------------- shared edge-tile body ----------------
    SUB = set(_os.environ.get("GAT_SUB", "gather,oh,er,ee,mm,epi").split(","))

    def edge_tile(t, pools, TL, TH, recs_dram, er_sb, il_sb, ih_sb, dl_sb):
        ep, pool_ups, pool_oht, pool_mis = pools
        T = TL + TH
        ert = er_sb[:, t, :]
        il = il_sb[:, t, :]
        ih = ih_sb[:, t, :]
        dl = dl_sb[:, t, :]

        g = ep.tile([128, T, RECW], dt.float16, tag="g")
        if "gather" in SUB:
            nc.gpsimd.dma_gather(
                out_ap=g[:, 0:TL, :], in_ap=recs_dram[0:SPLIT, :],
                idxs_ap=il, num_idxs=TL * 128, num_idxs_reg=TL * 128,
                elem_size=RECW, single_packet=False)
            nc.gpsimd.dma_gather(
                out_ap=g[:, TL:T, :], in_ap=recs_dram[SPLIT:NP, :],
                idxs_ap=ih, num_idxs=TH * 128, num_idxs_reg=TH * 128,
                elem_size=RECW, single_packet=False)
        else:
            nc.vector.memset(g[:], 0.125)

        ohs = ep.tile([128, T, 128], dt.float16, tag="ohs")
        erps = pool_mis.tile([128, T * 4], dt.float32, tag="erps", space="PSUM")
        if "oh" in SUB:
            for j in range(T):
                nc.vector.tensor_scalar(
                    out=ohs[:, j, :], in0=iotaF[:], scalar1=dl[:, j:j + 1],
                    scalar2=None, op0=OP.is_equal)
        else:
            nc.vector.memset(ohs[:], 0.0)
        if "er" in SUB:
            for j0 in range(0, T, 2):
                jn = min(2, T - j0)
                ohT_ps = pool_oht.tile([128, 2, 128], dt.float16, tag="ohT_ps")
                for jj in range(jn):
                    nc.tensor.transpose(ohT_ps[:, jj, :],
                                        ohs[:, j0 + jj, :], ident16[:])
                ohT = ep.tile([128, 2, 128], dt.float16, tag="ohT")
                nc.scalar.copy(ohT[:, 0:jn, :], ohT_ps[:, 0:jn, :])
                for jj in range(jn):
                    j = j0 + jj
                    nc.tensor.matmul(out=erps[:, j * 4:(j + 1) * 4],
                                     lhsT=ohT[:, jj, :],
                                     rhs=ert, start=True, stop=True)

        if "ee" in SUB:
            lx = ep.tile([128, T * 4], dt.float32, tag="lx")
            nc.vector.tensor_tensor(
                out=lx[:].rearrange("p (t f) -> p t f", f=4),
                in0=g[:, :, 0:4],
                in1=erps[:].rearrange("p (t f) -> p t f", f=4),
                op=OP.add)
            nc.vector.scalar_tensor_tensor(
                out=lx[:], in0=lx[:], scalar=NEG, in1=lx[:],
                op0=OP.mult, op1=OP.max)
            nc.scalar.activation(
                g[:, :, 4:8], lx[:].rearrange("p (t f) -> p t f", f=4), AF.Exp)

        Ups = pool_ups.tile([128, 4 + D1], dt.float32, tag="Ups", space="PSUM")
        if "mm" in SUB:
            nc.vector.tensor_tensor(
                out=g[:, :, 8:8 + D1].rearrange("p t (h d) -> p t h d", h=HEADS),
                in0=g[:, :, 8:8 + D1].rearrange("p t (h d) -> p t h d", h=HEADS),
                in1=g[:, :, 4:8].unsqueeze(-1).broadcast_to([128, T, HEADS, HID]),
                op=OP.mult)
            for j in range(T):
                nc.tensor.matmul(out=Ups[:], lhsT=ohs[:, j, :],
                                 rhs=g[:, j, 4:8 + D1],
                                 start=(j == 0), stop=(j == T - 1))
        else:
            zmm = ep.tile([128, 4 + D1], dt.float32, tag="zmm")
            nc.vector.memset(zmm[:], 1.0)
            nc.tensor.matmul(out=Ups[:], lhsT=ident32[:],
                             rhs=zmm[:], start=True, stop=True)
        return Ups

    # ---------------- Layer 1 edge phase + layer-2 record build ----------
    if not MERGE:
        _tc_close()
    if "l1" in phases:
      if not MERGE:
          _tc_ctx = TileContext(nc)
          tc = _tc_ctx.__enter__()
          _tc_open[0] = True
      if True:
        with (tc.tile_pool(name="ep1", bufs=3) as ep,
              tc.tile_pool(name="ups1", bufs=2, space="PSUM") as pool_ups,
              tc.tile_pool(name="oht1", bufs=2, space="PSUM") as pool_oht,
              tc.tile_pool(name="mis1", bufs=1, space="PSUM") as pool_mis):
            pools = (ep, pool_ups, pool_oht, pool_mis)
            for t in range(TILES):
                Ups = edge_tile(t, pools, T1L, T1H, recs1, er1_sb,
                                il1_sb, ih1_sb, dl1_sb)
                r0, r1 = t * 128, (t + 1) * 128
                if "epi" not in SUB:
                    dr = ep.tile([128, 4], dt.float32, tag="dr")
                    nc.vector.tensor_copy(dr[:], Ups[:, 0:4])
                    nc.vector.tensor_copy(dr[:], dr[:])
                    continue
                s = ep.tile([128, 4], dt.float32, tag="s")
                nc.vector.tensor_scalar_max(s[:], Ups[:, 0:4], 1e-30)
                rs = ep.tile([128, 4], dt.float32, tag="rs")
                nc.vector.reciprocal(rs[:], s[:])
                x1 = ep.tile([128, D1], dt.float32, tag="x1")
                nc.vector.tensor_tensor(
                    out=x1[:].rearrange("p (h d) -> p h d", h=HEADS),
                    in0=Ups[:, 4:4 + D1].rearrange("p (h d) -> p h d", h=HEADS),
                    in1=rs[:].unsqueeze(-1).broadcast_to([128, HEADS, HID]),
                    op=OP.mult)
                nc.vector.tensor_tensor(x1[:], x1[:], b1sb[:], op=OP.add)
                # h = elu(x) = relu(x) + exp(min(x,0)) - 1, min/relu/exp on ACT
                rn = ep.tile([128, D1], dt.float32, tag="rn")
                nc.scalar.activation(rn[:], x1[:], AF.Relu, scale=-1.0)
                ex = ep.tile([128, D1], dt.float32, tag="ex")
                nc.scalar.activation(ex[:], rn[:], AF.Exp, scale=-1.0)
                rp = ep.tile([128, D1], dt.float32, tag="rp")
                nc.scalar.activation(rp[:], x1[:], AF.Relu)
                hp = ep.tile([128, D1], dt.float32, tag="hp")
                nc.vector.scalar_tensor_tensor(
                    out=hp[:], in0=ex[:], scalar=-1.0, in1=rp[:],
                    op0=OP.add, op1=OP.add)
                hT = ep.tile([128, 256], dt.float16, tag="hT")
                for half in range(2):
                    tp = pool_oht.tile([128, 128], dt.float32, tag="tp",
                                       space="PSUM", bufs=1)
                    nc.tensor.transpose(
                        tp[:], hp[:, half * 128:(half + 1) * 128], ident32[:])
                    nc.scalar.copy(hT[:, half * 128:(half + 1) * 128], tp[:])
                z2ps = pool_mis.tile([128, D2], dt.float32, tag="z2ps",
                                     space="PSUM")
                nc.tensor.matmul(out=z2ps[:], lhsT=hT[:, 0:128], rhs=W2sb0[:],
                                 start=True, stop=False)
                nc.tensor.matmul(out=z2ps[:], lhsT=hT[:, 128:256], rhs=W2sb1[:],
                                 start=False, stop=True)
                e2ps = pool_mis.tile([128, 8], dt.float32, tag="e2ps",
                                     space="PSUM")
                nc.tensor.matmul(out=e2ps[:], lhsT=hT[:, 0:128], rhs=cw2sb0[:],
                                 start=True, stop=False)
                nc.tensor.matmul(out=e2ps[:], lhsT=hT[:, 128:256], rhs=cw2sb1[:],
                                 start=False, stop=True)
                rec2 = ep.tile([128, RECW], dt.float16, tag="rec2")
                nc.scalar.copy(rec2[:, 8:8 + D2], z2ps[:])
                nc.vector.tensor_copy(rec2[:, 0:4], e2ps[:, 0:4])
                nc.vector.tensor_copy(er2_sb[:, t, :], e2ps[:, 4:8])
                nc.sync.dma_start(recs2s[r0:r1, :], rec2[:])
                if "ag" in phases and t == 23:
                    nc.gpsimd.collective_compute(
                        kind="AllGather", op=OP.bypass,
                        replica_groups=[list(range(NCORES))],
                        ins=[recs2s[0:24 * 128, :]],
                        outs=[recs2f[0:NCORES * 24 * 128, :]])
            if "ag" in phases:
                nc.gpsimd.collective_compute(
                    kind="AllGather", op=OP.bypass,
                    replica_groups=[list(range(NCORES))],
                    ins=[recs2s[24 * 128:SH, :]],
                    outs=[recs2f[NCORES * 24 * 128:NP, :]])

    # ---------------- Layer 2 edge phase + output ----------------
    if "l1" in phases and not MERGE:
        _tc_close()
    if "l2" in phases:
      if not MERGE:
          _tc_ctx = TileContext(nc)
          tc = _tc_ctx.__enter__()
          _tc_open[0] = True
      if True:
        with (tc.tile_pool(name="ep2", bufs=3) as ep,
              tc.tile_pool(name="ups2", bufs=2, space="PSUM") as pool_ups,
              tc.tile_pool(name="oht2", bufs=2, space="PSUM") as pool_oht,
              tc.tile_pool(name="mis2", bufs=1, space="PSUM") as pool_mis):
            pools = (ep, pool_ups, pool_oht, pool_mis)
            for t in range(TILES):
                Ups = edge_tile(t, pools, T2L, T2H, recs2f, er2_sb,
                                il2_sb, ih2_sb, dl2_sb)
                r0, r1 = t * 128, (t + 1) * 128
                s = ep.tile([128, 4], dt.float32, tag="s")
                nc.vector.tensor_scalar_max(s[:], Ups[:, 0:4], 1e-30)
                rs = ep.tile([128, 4], dt.float32, tag="rs")
                nc.vector.reciprocal(rs[:], s[:])
                u = ep.tile([128, D2], dt.float32, tag="u")
                nc.vector.tensor_tensor(
                    out=u[:].rearrange("p (h d) -> p h d", h=HEADS),
                    in0=Ups[:, 4:4 + D2].rearrange("p (h d) -> p h d", h=HEADS),
                    in1=rs[:].unsqueeze(-1).broadcast_to([128, HEADS, OUT]),
                    op=OP.mult)
                red = ep.tile([128, OUT], dt.float32, tag="red")
                nc.vector.tensor_reduce(
                    out=red[:],
                    in_=u[:].rearrange("p (h d) -> p h d", h=HEADS)
                            .transpose([0, 2, 1]),
                    axis=mybir.AxisListType.X, op=OP.add)
                nc.vector.scalar_tensor_tensor(
                    out=out_sb[:, t, :], in0=red[:], scalar=1.0 / HEADS,
                    in1=b2msb[:], op0=OP.mult, op1=OP.add)
            nc.sync.dma_start(
                out[:].rearrange("(t p) w -> p t w", p=128), out_sb[:])

    _tc_close()
    nc.finalize()
    return nc


_CACHE = {}


def kernel(x, src, dst, W1, al1, ar1, b1, W2, al2, ar2, b2):
    from concourse.bass_utils import run_bass_kernel_spmd

    args = [np.asarray(a) for a in
            (x, src, dst, W1, al1, ar1, b1, W2, al2, ar2, b2)]
    consts, per_core, meta = _host_prep(*args)
    import os
    phases = tuple(os.environ.get("GAT_PHASES", "p0,l1,ag,l2").split(","))
    key = (meta["T1L"], meta["T1H"], meta["T2L"], meta["T2H"], phases)
    if key not in _CACHE:
        _CACHE[key] = _build_kernel(*key[:4], phases=phases)
    nc = _CACHE[key]

    ncores_run = NCORES if "ag" in phases else int(os.environ.get("GAT_NCORES", NCORES))
    in_maps = [{**consts, **per_core[c]} for c in range(ncores_run)]
    res = run_bass_kernel_spmd(nc, in_maps, core_ids=list(range(ncores_run)))
    global _LAST_RESULT
    _LAST_RESULT = res

    out_full = np.zeros((N, OUT), np.float32)
    node_of_slot = meta["node_of_slot"]
    for c in range(ncores_run):
        shard = res.results[c]["out"]
        valid = node_of_slot[c] >= 0
        out_full[node_of_slot[c][valid]] = shard[valid]
    return out_full

